# revision 13
# baseline (speedup 1.0000x reference)
"""Triplane embedding-lookup + MLP kernel for Trainium2 (8 NeuronCores), v3.

Architecture (vs v2 baseline at ~1.37ms):
  - Host: bucket-sort points by (band(y0(c1)) -> core, band(y0(c2)) -> bucket),
    precompute window-local int16 cell indices (3 per point) and grouped
    bilinear weights (6 bf16 per point) on host.  Patch tables are stored in a
    finite-difference basis [v00, dx, dy, dxy] per cell, bf16 (256B rows).
  - Device per block (4096 pts): 6 SWDGE dma_gathers (2048 idxs, 256B elem),
    grouped-weight combine on DVE in bf16 (17 ops, exploits shared fractional
    parts across the 3 planes: feats = V0 + f0*A + f1*B + f2*C + f01*Dxy0 +
    f12*Dxy1 + f02*Dxy2), PE transpose per 128-pt group into PSUM, bias-folded
    K=33 L0 matmul, N=1024 bf16 matmul MLP with |w3| folded into W2's rows and
    sign(w3) as the M=1 output matmul, relu/evac split across ACT+DVE.
  - Host unsorts the output.
"""

import sys

sys.path.insert(0, "/opt/trn_rl_repo")

from contextlib import ExitStack

import numpy as np

RES = 512
CELLS = RES * RES
EMB = 32
HID = 128
N = 1_000_000
NCORES = 8

BROWS = 64           # plane rows per band
WIN = BROWS * RES    # 32768 rows per window (int16-addressable)
BCAP = 16384         # points per bucket (padded)
NPB = 8              # buckets per core
NP = NPB * BCAP      # 131072 points per core
NB = 4096            # points per block
KB = NB // 128       # 32 k-groups per block
NBLK = NP // NB      # 32 blocks per core
GQ = 1024            # idxs per dma_gather (65 descs/lane fits the SWDGE ring;
                     # 2048 -> 129/lane deadlocks the carveout ring on HW)
S = NB // 16         # 256 idx cols per plane per block

LAST_RESULTS = None
_BUILT = {}


def _build_nc(do_finalize: bool = True):
    from concourse import bacc, mybir
    import concourse.tile as tile
    from concourse.masks import make_identity

    dt = mybir.dt
    f32 = dt.float32
    i16 = dt.int16
    bf16 = dt.bfloat16
    add = mybir.AluOpType.add
    mult = mybir.AluOpType.mult
    amax = mybir.AluOpType.max
    AF = mybir.ActivationFunctionType

    nc = bacc.Bacc("TRN2", target_bir_lowering=False, num_swdge_queues=4)

    pt0d = nc.dram_tensor("pt0", [WIN, 128], bf16, kind="ExternalInput")
    pt1d = nc.dram_tensor("pt1", [NPB * WIN, 128], bf16, kind="ExternalInput")
    pt2d = nc.dram_tensor("pt2", [NPB * WIN, 128], bf16, kind="ExternalInput")
    idxd = nc.dram_tensor("idx", [128, NBLK * 3 * S], i16, kind="ExternalInput")
    wd = nc.dram_tensor("wgt", [128, NBLK * KB * 12], bf16, kind="ExternalInput")
    w0d = nc.dram_tensor("w0c", [33, HID], bf16, kind="ExternalInput")
    w1d = nc.dram_tensor("w1c", [HID, HID], bf16, kind="ExternalInput")
    w2d = nc.dram_tensor("w2c", [HID, HID], bf16, kind="ExternalInput")
    s3d = nc.dram_tensor("s3c", [HID, 1], bf16, kind="ExternalInput")
    b1d = nc.dram_tensor("b1c", [HID, 1], f32, kind="ExternalInput")
    b2d = nc.dram_tensor("b2c", [HID, 1], f32, kind="ExternalInput")
    b3d = nc.dram_tensor("b3c", [1, 1], f32, kind="ExternalInput")
    outd = nc.dram_tensor("out", [NP], bf16, kind="ExternalOutput")
    outv = outd[:].unsqueeze(0)

    with tile.TileContext(nc) as tc, ExitStack() as ctx:
        cpool = ctx.enter_context(tc.tile_pool(name="consts", bufs=1))

        def ct(shape, dtp, tag):
            return cpool.tile(shape, dtp, tag=tag, name=tag)

        w0s = ct([33, HID], bf16, "w0s")
        w1s = ct([HID, HID], bf16, "w1s")
        w2s = ct([HID, HID], bf16, "w2s")
        s3s = ct([HID, 1], bf16, "s3s")
        b1s = ct([HID, 1], f32, "b1s")
        b2s = ct([HID, 1], f32, "b2s")
        b3s = ct([1, 1], f32, "b3s")
        ident = ct([128, 128], bf16, "ident")
        for s, d in ((w0s, w0d), (w1s, w1d), (w2s, w2d), (s3s, s3d),
                     (b1s, b1d), (b2s, b2d), (b3s, b3d)):
            nc.sync.dma_start(s[:], d[:])
        make_identity(nc, ident[:])

        io = ctx.enter_context(tc.tile_pool(name="io", bufs=3))
        gpool = ctx.enter_context(tc.tile_pool(name="gather", bufs=2))
        work = ctx.enter_context(tc.tile_pool(name="work", bufs=2))
        spool = ctx.enter_context(tc.tile_pool(name="stage", bufs=2))
        psum = ctx.enter_context(tc.tile_pool(name="psum", bufs=2, space="PSUM"))

        # fts buffers carry a constant ones-row (row 32) for the K=33
        # bias-folded L0; write it once per physical buffer before the loop.
        for _ in range(2):
            f = spool.tile([33, 1024], bf16, tag="fts", name="fts", bufs=2)
            nc.vector.memset(f[32:33, :], 1.0)

        for b in range(NBLK):
            j = b // (BCAP // NB)

            idxt = io.tile([128, 3 * S], i16, tag="idxt", name="idxt", bufs=3)
            nc.sync.dma_start(idxt[:], idxd[:, b * 3 * S:(b + 1) * 3 * S])
            wt = io.tile([128, KB * 12], bf16, tag="wt", name="wt", bufs=3)
            nc.sync.dma_start(wt[:], wd[:, b * KB * 12:(b + 1) * KB * 12])
            wt4 = wt[:].rearrange("p (k c t) -> p k c t", c=6, t=2)

            srcs = (pt0d[:],
                    pt1d[:][j * WIN:(j + 1) * WIN, :],
                    pt2d[:][j * WIN:(j + 1) * WIN, :])
            gs = []
            for pl in range(3):
                g = gpool.tile([128, KB, 128], bf16, tag=f"g{pl}",
                               name=f"g{pl}", bufs=3)
                for h in range(4):
                    nc.gpsimd.dma_gather(
                        out_ap=g[:, h * (KB // 4):(h + 1) * (KB // 4), :],
                        in_ap=srcs[pl],
                        idxs_ap=idxt[:, pl * S + h * (S // 4):
                                     pl * S + (h + 1) * (S // 4)],
                        num_idxs=GQ, num_idxs_reg=GQ, elem_size=128,
                        queue_num=(12 * b + 4 * pl + h) % 4)
                gs.append(g)

            # grouped-weight combine: each gathered row is [v00, dx, dy, dxy]
            # blocks of 32ch. feats = (v0_0+v0_1+v0_2) + f0*(dx0+dx2)
            #   + f1*(dx1+dy0) + f2*(dy1+dy2) + f01*dxy0 + f12*dxy1 + f02*dxy2
            def blk(g, i):
                return g[:, :, i * 32:(i + 1) * 32]

            def blk4(g, i):
                # same 32ch slice viewed as 16 aligned bf16 pairs
                return g[:, :, i * 32:(i + 1) * 32].rearrange(
                    "p k (a t) -> p k a t", t=2)

            def wb(c):
                # [128, KB, 1->16, 2]: stride-0 over the 16 pair-groups but a
                # step-1 innermost pair, keeping the 2x DVE mode
                return wt4[:, :, c, :].unsqueeze(2).to_broadcast(
                    [128, KB, 16, 2])

            tA = work.tile([128, KB, 32], bf16, tag="tA", name="tA")
            nc.vector.tensor_tensor(out=tA[:], in0=blk(gs[0], 1),
                                    in1=blk(gs[2], 1), op=add)
            tB = work.tile([128, KB, 32], bf16, tag="tB", name="tB")
            nc.vector.tensor_tensor(out=tB[:], in0=blk(gs[1], 1),
                                    in1=blk(gs[0], 2), op=add)
            tC = work.tile([128, KB, 32], bf16, tag="tC", name="tC")
            nc.vector.tensor_tensor(out=tC[:], in0=blk(gs[1], 2),
                                    in1=blk(gs[2], 2), op=add)
            acc = work.tile([128, KB, 32], bf16, tag="acc", name="acc")
            nc.vector.tensor_tensor(out=acc[:], in0=blk(gs[0], 0),
                                    in1=blk(gs[1], 0), op=add)
            nc.vector.tensor_tensor(out=acc[:], in0=acc[:],
                                    in1=blk(gs[2], 0), op=add)
            terms = ((tA[:].rearrange("p k (a t) -> p k a t", t=2), 0),
                     (tB[:].rearrange("p k (a t) -> p k a t", t=2), 1),
                     (tC[:].rearrange("p k (a t) -> p k a t", t=2), 2),
                     (blk4(gs[0], 3), 3), (blk4(gs[1], 3), 4),
                     (blk4(gs[2], 3), 5))
            for src, c in terms:
                p = work.tile([128, KB, 16, 2], bf16, tag="p", name="p",
                              bufs=3)
                nc.vector.tensor_tensor(out=p[:], in0=src, in1=wb(c), op=mult)
                nc.vector.tensor_tensor(
                    out=acc[:], in0=acc[:],
                    in1=p[:].rearrange("p k a t -> p k (a t)"), op=add)
            feats3 = acc[:]

            res = spool.tile([1, NB], bf16, tag="res", name="res", bufs=2)
            for hb in range(4):
                ftp = psum.tile([33, 1024], bf16, tag="ftp", name="ftp",
                                space="PSUM", bufs=2)
                for q in range(8):
                    k = hb * 8 + q
                    nc.tensor.transpose(
                        out=ftp[0:32, q * 128:(q + 1) * 128],
                        in_=feats3[:, k, :], identity=ident[:])
                fts = spool.tile([33, 1024], bf16, tag="fts", name="fts",
                                 bufs=2)
                nc.scalar.activation(fts[0:32, :], ftp[0:32, :], AF.Copy)

                for s in range(2):
                    fsl = fts[:, s * 512:(s + 1) * 512]
                    mm0 = psum.tile([HID, 512], f32, tag="mm", name="mm",
                                    space="PSUM", bufs=4)
                    nc.tensor.matmul(out=mm0[:], lhsT=w0s[:], rhs=fsl,
                                     start=True, stop=True)
                    h0 = work.tile([HID, 512], bf16, tag="h0", name="h0",
                                   bufs=3)
                    nc.scalar.activation(h0[:], mm0[:], AF.Relu)

                    mm1 = psum.tile([HID, 512], f32, tag="mm", name="mm",
                                    space="PSUM", bufs=4)
                    nc.tensor.matmul(out=mm1[:], lhsT=w1s[:], rhs=h0[:],
                                     start=True, stop=True)
                    h1 = work.tile([HID, 512], bf16, tag="h1", name="h1",
                                   bufs=3)
                    nc.scalar.activation(h1[:], mm1[:], AF.Relu,
                                         bias=b1s[:, 0:1])

                    mm2 = psum.tile([HID, 512], f32, tag="mm", name="mm",
                                    space="PSUM", bufs=4)
                    nc.tensor.matmul(out=mm2[:], lhsT=w2s[:], rhs=h1[:],
                                     start=True, stop=True)
                    h2w = work.tile([HID, 512], bf16, tag="h2w", name="h2w",
                                    bufs=3)
                    nc.vector.tensor_scalar(out=h2w[:], in0=mm2[:],
                                            scalar1=b2s[:, 0:1], scalar2=0.0,
                                            op0=add, op1=amax)

                    mm3 = psum.tile([1, 512], f32, tag="mm3", name="mm3",
                                    space="PSUM", bufs=2)
                    nc.tensor.matmul(out=mm3[:], lhsT=s3s[:], rhs=h2w[:],
                                     start=True, stop=True)
                    o0 = hb * 1024 + s * 512
                    if s == 0:
                        nc.vector.tensor_scalar_add(res[0:1, o0:o0 + 512],
                                                    mm3[:], b3s[0:1, 0:1])
                    else:
                        nc.scalar.activation(res[0:1, o0:o0 + 512], mm3[:],
                                             AF.Identity, bias=b3s[0:1, 0:1])

            nc.sync.dma_start(outv[:, b * NB:(b + 1) * NB], res[:])

    if do_finalize:
        nc.finalize()
    return nc


def _get_nc():
    if "nc" not in _BUILT:
        _BUILT["nc"] = _build_nc()
    return _BUILT["nc"]


def _build_patch_tables(planes: np.ndarray) -> np.ndarray:
    # planes [3, 32, 512, 512] -> finite-difference patch tables
    # PT[pl, cell, 4*32] bf16 with blocks [v00, dx, dy, dxy].
    import ml_dtypes

    p = planes.transpose(0, 2, 3, 1).astype(np.float32)  # [3, H, W, C]
    v01 = np.zeros_like(p)
    v01[:, :, :-1] = p[:, :, 1:]
    v10 = np.zeros_like(p)
    v10[:, :-1, :] = p[:, 1:, :]
    v11 = np.zeros_like(p)
    v11[:, :-1, :-1] = p[:, 1:, 1:]
    pt = np.empty((3, RES, RES, 4, EMB), dtype=np.float32)
    pt[:, :, :, 0] = p
    pt[:, :, :, 1] = v01 - p
    pt[:, :, :, 2] = v10 - p
    pt[:, :, :, 3] = v11 - v10 - v01 + p
    return np.ascontiguousarray(
        pt.reshape(3, CELLS, 4 * EMB)).astype(ml_dtypes.bfloat16)


def _prepare(inputs):
    import ml_dtypes

    bf = ml_dtypes.bfloat16
    coords = np.asarray(inputs["coordinates"], dtype=np.float32)
    n = coords.shape[0]

    pt = _build_patch_tables(np.asarray(inputs["planes"], np.float32))

    w0 = np.asarray(inputs["w0"], np.float32)
    w1 = np.asarray(inputs["w1"], np.float32)
    w2 = np.asarray(inputs["w2"], np.float32)
    w3 = np.asarray(inputs["w3"], np.float32).reshape(HID)
    b0 = np.asarray(inputs["b0"], np.float32)
    b1 = np.asarray(inputs["b1"], np.float32).reshape(HID, 1)
    b2 = np.asarray(inputs["b2"], np.float32).reshape(HID)
    b3 = np.asarray(inputs["b3"], np.float32).reshape(1, 1)

    w0c = np.ascontiguousarray(
        np.vstack([w0.T, b0.reshape(1, HID)])).astype(bf)   # [33, 128]
    w1c = np.ascontiguousarray(w1.T).astype(bf)
    w2p = w2 * np.abs(w3).reshape(HID, 1)                   # scale rows
    w2c = np.ascontiguousarray(w2p.T).astype(bf)
    s3c = np.sign(w3).reshape(HID, 1).astype(bf)
    b2c = (b2 * np.abs(w3)).reshape(HID, 1).astype(np.float32)

    # pixel coords, floors, fracs (mirrors reference: x = (c+1)*0.5*(R-1))
    pix = (coords + 1.0) * np.float32(0.5 * (RES - 1))
    p0 = np.floor(pix).astype(np.int64)
    fr = (pix - p0.astype(np.float32)).astype(np.float32)
    p0 = np.clip(p0, 0, RES - 1)

    gband = np.clip(p0[:, 1] >> 6, 0, NCORES - 1)
    jband = np.clip(p0[:, 2] >> 6, 0, NPB - 1)
    key = gband * NPB + jband
    order = np.argsort(key, kind="stable")
    counts = np.bincount(key, minlength=NCORES * NPB)
    assert counts.max() <= BCAP, f"bucket overflow: {counts.max()}"

    # per-point window-local indices and grouped weights, in original order
    idx0 = (p0[:, 1] - (gband << 6)) * RES + p0[:, 0]
    idx1 = (p0[:, 2] - (jband << 6)) * RES + p0[:, 1]
    idx2 = (p0[:, 2] - (jband << 6)) * RES + p0[:, 0]
    idx_all = np.stack([idx0, idx1, idx2], axis=1).astype(np.int16)
    f0, f1, f2 = fr[:, 0], fr[:, 1], fr[:, 2]
    # each weight duplicated x2 so the DVE multiply's in1 stream has an
    # innermost step-1 pair (keeps the bf16 2x perf mode; a pure stride-0
    # broadcast demotes tensor_tensor to 1x)
    w_all = np.repeat(np.stack([f0, f1, f2, f0 * f1, f1 * f2, f0 * f2],
                               axis=1), 2, axis=1).astype(bf)

    starts = np.concatenate(([0], np.cumsum(counts)))
    idx_s = np.zeros((NCORES * NPB, BCAP, 3), np.int16)
    w_s = np.zeros((NCORES * NPB, BCAP, 12), bf)
    ids = np.full((NCORES * NPB, BCAP), -1, np.int64)
    for kbkt in range(NCORES * NPB):
        sel = order[starts[kbkt]:starts[kbkt + 1]]
        idx_s[kbkt, :len(sel)] = idx_all[sel]
        w_s[kbkt, :len(sel)] = w_all[sel]
        ids[kbkt, :len(sel)] = sel

    in_maps = []
    for g in range(NCORES):
        ci = idx_s[g * NPB:(g + 1) * NPB].reshape(NP, 3)
        # [NBLK, 3, S, 16] -> wrap 16 partitions, tile x8 -> [128, NBLK*3*S]
        I = ci.reshape(NBLK, NB, 3).transpose(0, 2, 1)         # [NBLK, 3, NB]
        I = I.reshape(NBLK, 3, S, 16).transpose(0, 1, 3, 2)    # [NBLK,3,16,S]
        I = np.tile(I, (1, 1, 8, 1)).transpose(2, 0, 1, 3)     # [128,NBLK,3,S]
        I = np.ascontiguousarray(I).reshape(128, NBLK * 3 * S)

        cw = w_s[g * NPB:(g + 1) * NPB].reshape(NP, 12)
        W = cw.reshape(NBLK, KB, 128, 12).transpose(2, 0, 1, 3)
        W = np.ascontiguousarray(W).reshape(128, NBLK * KB * 12)

        in_maps.append({
            "pt0": np.ascontiguousarray(pt[0][g * WIN:(g + 1) * WIN]),
            "pt1": pt[1], "pt2": pt[2],
            "idx": I, "wgt": W,
            "w0c": w0c, "w1c": w1c, "w2c": w2c, "s3c": s3c,
            "b1c": b1, "b2c": b2c, "b3c": b3,
        })
    return in_maps, ids.reshape(NCORES, NP), n


def kernel(**inputs: np.ndarray) -> np.ndarray:
    global LAST_RESULTS
    from concourse.bass_utils import run_bass_kernel_spmd

    in_maps, flat_ids, n = _prepare(inputs)
    nc = _get_nc()
    LAST_RESULTS = run_bass_kernel_spmd(nc, in_maps, list(range(NCORES)))

    full = np.zeros(n, np.float32)
    for g in range(NCORES):
        o = np.asarray(LAST_RESULTS.results[g]["out"]).astype(np.float32)
        m = flat_ids[g] >= 0
        full[flat_ids[g][m]] = o[m]
    return full.reshape(1, n, 1).astype(np.float32)


# revision 14
# speedup vs baseline: 1.4244x; 1.4244x over previous
"""Triplane embedding-lookup + MLP kernel for Trainium2 (8 NeuronCores), v3.

Architecture (vs v2 baseline at ~1.37ms):
  - Host: bucket-sort points by (band(y0(c1)) -> core, band(y0(c2)) -> bucket),
    precompute window-local int16 cell indices (3 per point) and grouped
    bilinear weights (6 bf16 per point) on host.  Patch tables are stored in a
    finite-difference basis [v00, dx, dy, dxy] per cell, bf16 (256B rows).
  - Device per block (4096 pts): 6 SWDGE dma_gathers (2048 idxs, 256B elem),
    grouped-weight combine on DVE in bf16 (17 ops, exploits shared fractional
    parts across the 3 planes: feats = V0 + f0*A + f1*B + f2*C + f01*Dxy0 +
    f12*Dxy1 + f02*Dxy2), PE transpose per 128-pt group into PSUM, bias-folded
    K=33 L0 matmul, N=1024 bf16 matmul MLP with |w3| folded into W2's rows and
    sign(w3) as the M=1 output matmul, relu/evac split across ACT+DVE.
  - Host unsorts the output.
"""

import sys

sys.path.insert(0, "/opt/trn_rl_repo")

from contextlib import ExitStack

import numpy as np

RES = 512
CELLS = RES * RES
EMB = 32
HID = 128
N = 1_000_000
NCORES = 8

BROWS = 64           # plane rows per band
WIN = BROWS * RES    # 32768 rows per window (int16-addressable)
BCAP = 16384         # points per bucket (padded)
NPB = 8              # buckets per core
NP = NPB * BCAP      # 131072 points per core
NB = 4096            # points per block
KB = NB // 128       # 32 k-groups per block
NBLK = NP // NB      # 32 blocks per core
GQ = 1024            # idxs per dma_gather (65 descs/lane fits the SWDGE ring;
                     # 2048 -> 129/lane deadlocks the carveout ring on HW)
S = NB // 16         # 256 idx cols per plane per block

LAST_RESULTS = None
_BUILT = {}


def _build_nc(do_finalize: bool = True):
    from concourse import bacc, mybir
    import concourse.tile as tile
    from concourse.masks import make_identity

    dt = mybir.dt
    f32 = dt.float32
    i16 = dt.int16
    bf16 = dt.bfloat16
    add = mybir.AluOpType.add
    mult = mybir.AluOpType.mult
    amax = mybir.AluOpType.max
    AF = mybir.ActivationFunctionType

    nc = bacc.Bacc("TRN2", target_bir_lowering=False, num_swdge_queues=4)

    pt0d = nc.dram_tensor("pt0", [WIN, 128], bf16, kind="ExternalInput")
    pt1d = nc.dram_tensor("pt1", [NPB * WIN, 128], bf16, kind="ExternalInput")
    pt2d = nc.dram_tensor("pt2", [NPB * WIN, 128], bf16, kind="ExternalInput")
    idxd = nc.dram_tensor("idx", [128, NBLK * 3 * S], i16, kind="ExternalInput")
    wd = nc.dram_tensor("wgt", [128, NBLK * KB * 6], bf16, kind="ExternalInput")
    w0d = nc.dram_tensor("w0c", [33, HID], bf16, kind="ExternalInput")
    w1d = nc.dram_tensor("w1c", [HID, HID], bf16, kind="ExternalInput")
    w2d = nc.dram_tensor("w2c", [HID, HID], bf16, kind="ExternalInput")
    s3d = nc.dram_tensor("s3c", [HID, 1], bf16, kind="ExternalInput")
    b1d = nc.dram_tensor("b1c", [HID, 1], f32, kind="ExternalInput")
    b2d = nc.dram_tensor("b2c", [HID, 1], f32, kind="ExternalInput")
    b3d = nc.dram_tensor("b3c", [1, 1], f32, kind="ExternalInput")
    outd = nc.dram_tensor("out", [NP], bf16, kind="ExternalOutput")
    outv = outd[:].unsqueeze(0)

    with tile.TileContext(nc) as tc, ExitStack() as ctx:
        cpool = ctx.enter_context(tc.tile_pool(name="consts", bufs=1))

        def ct(shape, dtp, tag):
            return cpool.tile(shape, dtp, tag=tag, name=tag)

        w0s = ct([33, HID], bf16, "w0s")
        w1s = ct([HID, HID], bf16, "w1s")
        w2s = ct([HID, HID], bf16, "w2s")
        s3s = ct([HID, 1], bf16, "s3s")
        b1s = ct([HID, 1], f32, "b1s")
        b2s = ct([HID, 1], f32, "b2s")
        b3s = ct([1, 1], f32, "b3s")
        ident = ct([128, 128], bf16, "ident")
        for s, d in ((w0s, w0d), (w1s, w1d), (w2s, w2d), (s3s, s3d),
                     (b1s, b1d), (b2s, b2d), (b3s, b3d)):
            nc.sync.dma_start(s[:], d[:])
        make_identity(nc, ident[:])

        io = ctx.enter_context(tc.tile_pool(name="io", bufs=3))
        gpool = ctx.enter_context(tc.tile_pool(name="gather", bufs=2))
        work = ctx.enter_context(tc.tile_pool(name="work", bufs=2))
        spool = ctx.enter_context(tc.tile_pool(name="stage", bufs=2))
        psum = ctx.enter_context(tc.tile_pool(name="psum", bufs=2, space="PSUM"))

        # fts buffers carry a constant ones-row (row 32) for the K=33
        # bias-folded L0; write it once per physical buffer before the loop.
        for _ in range(2):
            f = spool.tile([33, 1024], bf16, tag="fts", name="fts", bufs=2)
            nc.vector.memset(f[32:33, :], 1.0)

        for b in range(NBLK):
            j = b // (BCAP // NB)

            idxt = io.tile([128, 3 * S], i16, tag="idxt", name="idxt", bufs=3)
            nc.sync.dma_start(idxt[:], idxd[:, b * 3 * S:(b + 1) * 3 * S])
            wt = io.tile([128, KB * 6], bf16, tag="wt", name="wt", bufs=3)
            nc.sync.dma_start(wt[:], wd[:, b * KB * 6:(b + 1) * KB * 6])
            wt3 = wt[:].rearrange("p (k c) -> p k c", c=6)

            srcs = (pt0d[:],
                    pt1d[:][j * WIN:(j + 1) * WIN, :],
                    pt2d[:][j * WIN:(j + 1) * WIN, :])
            gs = []
            for pl in range(3):
                g = gpool.tile([128, KB, 128], bf16, tag=f"g{pl}",
                               name=f"g{pl}", bufs=2)
                for h in range(4):
                    nc.gpsimd.dma_gather(
                        out_ap=g[:, h * (KB // 4):(h + 1) * (KB // 4), :],
                        in_ap=srcs[pl],
                        idxs_ap=idxt[:, pl * S + h * (S // 4):
                                     pl * S + (h + 1) * (S // 4)],
                        num_idxs=GQ, num_idxs_reg=GQ, elem_size=128,
                        queue_num=(12 * b + 4 * pl + h) % 4)
                gs.append(g)

            # grouped-weight combine: each gathered row is [v00, dx, dy, dxy]
            # blocks of 32ch. feats = (v0_0+v0_1+v0_2) + f0*(dx0+dx2)
            #   + f1*(dx1+dy0) + f2*(dy1+dy2) + f01*dxy0 + f12*dxy1 + f02*dxy2
            def blk(g, i):
                return g[:, :, i * 32:(i + 1) * 32]

            def wb(c):
                return wt3[:, :, c].unsqueeze(2).to_broadcast([128, KB, 32])

            tA = work.tile([128, KB, 32], bf16, tag="tA", name="tA")
            nc.vector.tensor_tensor(out=tA[:], in0=blk(gs[0], 1),
                                    in1=blk(gs[2], 1), op=add)
            tB = work.tile([128, KB, 32], bf16, tag="tB", name="tB")
            nc.vector.tensor_tensor(out=tB[:], in0=blk(gs[1], 1),
                                    in1=blk(gs[0], 2), op=add)
            tC = work.tile([128, KB, 32], bf16, tag="tC", name="tC")
            nc.vector.tensor_tensor(out=tC[:], in0=blk(gs[1], 2),
                                    in1=blk(gs[2], 2), op=add)
            acc = work.tile([128, KB, 32], bf16, tag="acc", name="acc")
            nc.vector.tensor_tensor(out=acc[:], in0=blk(gs[0], 0),
                                    in1=blk(gs[1], 0), op=add)
            nc.vector.tensor_tensor(out=acc[:], in0=acc[:],
                                    in1=blk(gs[2], 0), op=add)
            terms = ((tA[:], 0), (tB[:], 1), (tC[:], 2),
                     (blk(gs[0], 3), 3), (blk(gs[1], 3), 4),
                     (blk(gs[2], 3), 5))
            for src, c in terms:
                p = work.tile([128, KB, 32], bf16, tag="p", name="p", bufs=3)
                nc.vector.tensor_tensor(out=p[:], in0=src, in1=wb(c), op=mult)
                nc.vector.tensor_tensor(out=acc[:], in0=acc[:], in1=p[:],
                                        op=add)
            feats3 = acc[:]

            res = spool.tile([1, NB], bf16, tag="res", name="res", bufs=2)
            for hb in range(4):
                ftp = psum.tile([33, 1024], bf16, tag="ftp", name="ftp",
                                space="PSUM", bufs=2)
                for q in range(8):
                    k = hb * 8 + q
                    nc.tensor.transpose(
                        out=ftp[0:32, q * 128:(q + 1) * 128],
                        in_=feats3[:, k, :], identity=ident[:])
                fts = spool.tile([33, 1024], bf16, tag="fts", name="fts",
                                 bufs=2)
                nc.scalar.activation(fts[0:32, :], ftp[0:32, :], AF.Copy)

                for s in range(2):
                    fsl = fts[:, s * 512:(s + 1) * 512]
                    mm0 = psum.tile([HID, 512], f32, tag="mm", name="mm",
                                    space="PSUM", bufs=4)
                    nc.tensor.matmul(out=mm0[:], lhsT=w0s[:], rhs=fsl,
                                     start=True, stop=True)
                    h0 = work.tile([HID, 512], bf16, tag="h0", name="h0",
                                   bufs=3)
                    nc.scalar.activation(h0[:], mm0[:], AF.Relu)

                    mm1 = psum.tile([HID, 512], f32, tag="mm", name="mm",
                                    space="PSUM", bufs=4)
                    nc.tensor.matmul(out=mm1[:], lhsT=w1s[:], rhs=h0[:],
                                     start=True, stop=True)
                    h1 = work.tile([HID, 512], bf16, tag="h1", name="h1",
                                   bufs=3)
                    nc.scalar.activation(h1[:], mm1[:], AF.Relu,
                                         bias=b1s[:, 0:1])

                    mm2 = psum.tile([HID, 512], f32, tag="mm", name="mm",
                                    space="PSUM", bufs=4)
                    nc.tensor.matmul(out=mm2[:], lhsT=w2s[:], rhs=h1[:],
                                     start=True, stop=True)
                    h2w = work.tile([HID, 512], bf16, tag="h2w", name="h2w",
                                    bufs=3)
                    nc.vector.tensor_scalar(out=h2w[:], in0=mm2[:],
                                            scalar1=b2s[:, 0:1], scalar2=0.0,
                                            op0=add, op1=amax)

                    mm3 = psum.tile([1, 512], f32, tag="mm3", name="mm3",
                                    space="PSUM", bufs=2)
                    nc.tensor.matmul(out=mm3[:], lhsT=s3s[:], rhs=h2w[:],
                                     start=True, stop=True)
                    o0 = hb * 1024 + s * 512
                    if s == 0:
                        nc.vector.tensor_scalar_add(res[0:1, o0:o0 + 512],
                                                    mm3[:], b3s[0:1, 0:1])
                    else:
                        nc.scalar.activation(res[0:1, o0:o0 + 512], mm3[:],
                                             AF.Identity, bias=b3s[0:1, 0:1])

            nc.sync.dma_start(outv[:, b * NB:(b + 1) * NB], res[:])

    if do_finalize:
        nc.finalize()
    return nc


def _get_nc():
    if "nc" not in _BUILT:
        _BUILT["nc"] = _build_nc()
    return _BUILT["nc"]


def _build_patch_tables(planes: np.ndarray) -> np.ndarray:
    # planes [3, 32, 512, 512] -> finite-difference patch tables
    # PT[pl, cell, 4*32] bf16 with blocks [v00, dx, dy, dxy].
    import ml_dtypes

    p = planes.transpose(0, 2, 3, 1).astype(np.float32)  # [3, H, W, C]
    v01 = np.zeros_like(p)
    v01[:, :, :-1] = p[:, :, 1:]
    v10 = np.zeros_like(p)
    v10[:, :-1, :] = p[:, 1:, :]
    v11 = np.zeros_like(p)
    v11[:, :-1, :-1] = p[:, 1:, 1:]
    pt = np.empty((3, RES, RES, 4, EMB), dtype=np.float32)
    pt[:, :, :, 0] = p
    pt[:, :, :, 1] = v01 - p
    pt[:, :, :, 2] = v10 - p
    pt[:, :, :, 3] = v11 - v10 - v01 + p
    return np.ascontiguousarray(
        pt.reshape(3, CELLS, 4 * EMB)).astype(ml_dtypes.bfloat16)


def _prepare(inputs):
    import ml_dtypes

    bf = ml_dtypes.bfloat16
    coords = np.asarray(inputs["coordinates"], dtype=np.float32)
    n = coords.shape[0]

    pt = _build_patch_tables(np.asarray(inputs["planes"], np.float32))

    w0 = np.asarray(inputs["w0"], np.float32)
    w1 = np.asarray(inputs["w1"], np.float32)
    w2 = np.asarray(inputs["w2"], np.float32)
    w3 = np.asarray(inputs["w3"], np.float32).reshape(HID)
    b0 = np.asarray(inputs["b0"], np.float32)
    b1 = np.asarray(inputs["b1"], np.float32).reshape(HID, 1)
    b2 = np.asarray(inputs["b2"], np.float32).reshape(HID)
    b3 = np.asarray(inputs["b3"], np.float32).reshape(1, 1)

    w0c = np.ascontiguousarray(
        np.vstack([w0.T, b0.reshape(1, HID)])).astype(bf)   # [33, 128]
    w1c = np.ascontiguousarray(w1.T).astype(bf)
    w2p = w2 * np.abs(w3).reshape(HID, 1)                   # scale rows
    w2c = np.ascontiguousarray(w2p.T).astype(bf)
    s3c = np.sign(w3).reshape(HID, 1).astype(bf)
    b2c = (b2 * np.abs(w3)).reshape(HID, 1).astype(np.float32)

    # pixel coords, floors, fracs (mirrors reference: x = (c+1)*0.5*(R-1))
    pix = (coords + 1.0) * np.float32(0.5 * (RES - 1))
    p0 = np.floor(pix).astype(np.int64)
    fr = (pix - p0.astype(np.float32)).astype(np.float32)
    p0 = np.clip(p0, 0, RES - 1)

    gband = np.clip(p0[:, 1] >> 6, 0, NCORES - 1)
    jband = np.clip(p0[:, 2] >> 6, 0, NPB - 1)
    key = gband * NPB + jband
    order = np.argsort(key, kind="stable")
    counts = np.bincount(key, minlength=NCORES * NPB)
    assert counts.max() <= BCAP, f"bucket overflow: {counts.max()}"

    # per-point window-local indices and grouped weights, in original order
    idx0 = (p0[:, 1] - (gband << 6)) * RES + p0[:, 0]
    idx1 = (p0[:, 2] - (jband << 6)) * RES + p0[:, 1]
    idx2 = (p0[:, 2] - (jband << 6)) * RES + p0[:, 0]
    idx_all = np.stack([idx0, idx1, idx2], axis=1).astype(np.int16)
    f0, f1, f2 = fr[:, 0], fr[:, 1], fr[:, 2]
    w_all = np.stack([f0, f1, f2, f0 * f1, f1 * f2, f0 * f2],
                     axis=1).astype(bf)

    starts = np.concatenate(([0], np.cumsum(counts)))
    idx_s = np.zeros((NCORES * NPB, BCAP, 3), np.int16)
    w_s = np.zeros((NCORES * NPB, BCAP, 6), bf)
    ids = np.full((NCORES * NPB, BCAP), -1, np.int64)
    for kbkt in range(NCORES * NPB):
        sel = order[starts[kbkt]:starts[kbkt + 1]]
        idx_s[kbkt, :len(sel)] = idx_all[sel]
        w_s[kbkt, :len(sel)] = w_all[sel]
        ids[kbkt, :len(sel)] = sel

    in_maps = []
    for g in range(NCORES):
        ci = idx_s[g * NPB:(g + 1) * NPB].reshape(NP, 3)
        # [NBLK, 3, S, 16] -> wrap 16 partitions, tile x8 -> [128, NBLK*3*S]
        I = ci.reshape(NBLK, NB, 3).transpose(0, 2, 1)         # [NBLK, 3, NB]
        I = I.reshape(NBLK, 3, S, 16).transpose(0, 1, 3, 2)    # [NBLK,3,16,S]
        I = np.tile(I, (1, 1, 8, 1)).transpose(2, 0, 1, 3)     # [128,NBLK,3,S]
        I = np.ascontiguousarray(I).reshape(128, NBLK * 3 * S)

        cw = w_s[g * NPB:(g + 1) * NPB].reshape(NP, 6)
        W = cw.reshape(NBLK, KB, 128, 6).transpose(2, 0, 1, 3)
        W = np.ascontiguousarray(W).reshape(128, NBLK * KB * 6)

        in_maps.append({
            "pt0": np.ascontiguousarray(pt[0][g * WIN:(g + 1) * WIN]),
            "pt1": pt[1], "pt2": pt[2],
            "idx": I, "wgt": W,
            "w0c": w0c, "w1c": w1c, "w2c": w2c, "s3c": s3c,
            "b1c": b1, "b2c": b2c, "b3c": b3,
        })
    return in_maps, ids.reshape(NCORES, NP), n


def kernel(**inputs: np.ndarray) -> np.ndarray:
    global LAST_RESULTS
    from concourse.bass_utils import run_bass_kernel_spmd

    in_maps, flat_ids, n = _prepare(inputs)
    nc = _get_nc()
    LAST_RESULTS = run_bass_kernel_spmd(nc, in_maps, list(range(NCORES)))

    full = np.zeros(n, np.float32)
    for g in range(NCORES):
        o = np.asarray(LAST_RESULTS.results[g]["out"]).astype(np.float32)
        m = flat_ids[g] >= 0
        full[flat_ids[g][m]] = o[m]
    return full.reshape(1, n, 1).astype(np.float32)


# revision 17
# speedup vs baseline: 1.6003x; 1.1234x over previous
"""Triplane embedding-lookup + MLP kernel for Trainium2 (8 NeuronCores), v3.

Architecture (vs v2 baseline at ~1.37ms):
  - Host: bucket-sort points by (band(y0(c1)) -> core, band(y0(c2)) -> bucket),
    precompute window-local int16 cell indices (3 per point) and grouped
    bilinear weights (6 bf16 per point) on host.  Patch tables are stored in a
    finite-difference basis [v00, dx, dy, dxy] per cell, bf16 (256B rows).
  - Device per block (4096 pts): 6 SWDGE dma_gathers (2048 idxs, 256B elem),
    grouped-weight combine on DVE in bf16 (17 ops, exploits shared fractional
    parts across the 3 planes: feats = V0 + f0*A + f1*B + f2*C + f01*Dxy0 +
    f12*Dxy1 + f02*Dxy2), PE transpose per 128-pt group into PSUM, bias-folded
    K=33 L0 matmul, N=1024 bf16 matmul MLP with |w3| folded into W2's rows and
    sign(w3) as the M=1 output matmul, relu/evac split across ACT+DVE.
  - Host unsorts the output.
"""

import sys

sys.path.insert(0, "/opt/trn_rl_repo")

from contextlib import ExitStack

import numpy as np

RES = 512
CELLS = RES * RES
EMB = 32
HID = 128
N = 1_000_000
NCORES = 8

BROWS = 64           # plane rows per band
WIN = BROWS * RES    # 32768 rows per window (int16-addressable)
BCAP = 16384         # points per bucket (padded)
NPB = 8              # buckets per core
NP = NPB * BCAP      # 131072 points per core
NB = 4096            # points per block
KB = NB // 128       # 32 k-groups per block
NBLK = NP // NB      # 32 blocks per core
GQ = 1024            # idxs per dma_gather (65 descs/lane fits the SWDGE ring;
                     # 2048 -> 129/lane deadlocks the carveout ring on HW)
S = NB // 16         # 256 idx cols per plane per block

LAST_RESULTS = None
_BUILT = {}


def _build_nc(do_finalize: bool = True):
    from concourse import bacc, mybir
    import concourse.tile as tile
    from concourse.masks import make_identity

    dt = mybir.dt
    f32 = dt.float32
    i16 = dt.int16
    bf16 = dt.bfloat16
    add = mybir.AluOpType.add
    mult = mybir.AluOpType.mult
    amax = mybir.AluOpType.max
    AF = mybir.ActivationFunctionType

    nc = bacc.Bacc("TRN2", target_bir_lowering=False, num_swdge_queues=4,
                   dynamic_dma_scratch_size=65536)

    pt0d = nc.dram_tensor("pt0", [WIN, 128], bf16, kind="ExternalInput")
    pt1d = nc.dram_tensor("pt1", [NPB * WIN, 128], bf16, kind="ExternalInput")
    pt2d = nc.dram_tensor("pt2", [NPB * WIN, 128], bf16, kind="ExternalInput")
    idxd = nc.dram_tensor("idx", [128, NBLK * 3 * S], i16, kind="ExternalInput")
    wd = nc.dram_tensor("wgt", [128, NBLK * KB * 6], bf16, kind="ExternalInput")
    w0d = nc.dram_tensor("w0c", [33, HID], bf16, kind="ExternalInput")
    w1d = nc.dram_tensor("w1c", [HID, HID], bf16, kind="ExternalInput")
    w2d = nc.dram_tensor("w2c", [HID, HID], bf16, kind="ExternalInput")
    s3d = nc.dram_tensor("s3c", [HID, 1], bf16, kind="ExternalInput")
    b1d = nc.dram_tensor("b1c", [HID, 1], f32, kind="ExternalInput")
    b2d = nc.dram_tensor("b2c", [HID, 1], f32, kind="ExternalInput")
    b3d = nc.dram_tensor("b3c", [1, 1], f32, kind="ExternalInput")
    outd = nc.dram_tensor("out", [NP], bf16, kind="ExternalOutput")
    outv = outd[:].unsqueeze(0)

    with tile.TileContext(nc) as tc, ExitStack() as ctx:
        cpool = ctx.enter_context(tc.tile_pool(name="consts", bufs=1))

        def ct(shape, dtp, tag):
            return cpool.tile(shape, dtp, tag=tag, name=tag)

        io = ctx.enter_context(tc.tile_pool(name="io", bufs=3))
        gpool = ctx.enter_context(tc.tile_pool(name="gather", bufs=2))
        work = ctx.enter_context(tc.tile_pool(name="work", bufs=2))
        spool = ctx.enter_context(tc.tile_pool(name="stage", bufs=2))
        psum = ctx.enter_context(tc.tile_pool(name="psum", bufs=2, space="PSUM"))

        def load_streams(b):
            it = io.tile([128, 3 * S], i16, tag="idxt", name="idxt", bufs=3)
            nc.sync.dma_start(it[:], idxd[:, b * 3 * S:(b + 1) * 3 * S])
            w = io.tile([128, KB * 6], bf16, tag="wt", name="wt", bufs=3)
            nc.sync.dma_start(w[:], wd[:, b * KB * 6:(b + 1) * KB * 6])
            return it, w

        # prefetch the first blocks' streams so the gather pipeline starts
        # before the (longer) const-load tail drains from the HWDGE queue
        pre = {b: load_streams(b) for b in range(2)}

        w0s = ct([33, HID], bf16, "w0s")
        w1s = ct([HID, HID], bf16, "w1s")
        w2s = ct([HID, HID], bf16, "w2s")
        s3s = ct([HID, 1], bf16, "s3s")
        b1s = ct([HID, 1], f32, "b1s")
        b2s = ct([HID, 1], f32, "b2s")
        b3s = ct([1, 1], f32, "b3s")
        ident = ct([128, 128], bf16, "ident")
        for s, d in ((w0s, w0d), (w1s, w1d), (w2s, w2d), (s3s, s3d),
                     (b1s, b1d), (b2s, b2d), (b3s, b3d)):
            nc.sync.dma_start(s[:], d[:])
        make_identity(nc, ident[:])

        # fts buffers carry a constant ones-row (row 32) for the K=33
        # bias-folded L0; write it once per physical buffer before the loop.
        for _ in range(2):
            f = spool.tile([33, 1024], bf16, tag="fts", name="fts", bufs=2)
            nc.vector.memset(f[32:33, :], 1.0)

        for b in range(NBLK):
            j = b // (BCAP // NB)

            idxt, wt = pre.pop(b) if b in pre else load_streams(b)
            wt3 = wt[:].rearrange("p (k c) -> p k c", c=6)

            srcs = (pt0d[:],
                    pt1d[:][j * WIN:(j + 1) * WIN, :],
                    pt2d[:][j * WIN:(j + 1) * WIN, :])
            gs = []
            for pl in range(3):
                g = gpool.tile([128, KB, 128], bf16, tag=f"g{pl}",
                               name=f"g{pl}", bufs=2)
                for h in range(4):
                    nc.gpsimd.dma_gather(
                        out_ap=g[:, h * (KB // 4):(h + 1) * (KB // 4), :],
                        in_ap=srcs[pl],
                        idxs_ap=idxt[:, pl * S + h * (S // 4):
                                     pl * S + (h + 1) * (S // 4)],
                        num_idxs=GQ, num_idxs_reg=GQ, elem_size=128,
                        queue_num=(12 * b + 4 * pl + h) % 4)
                gs.append(g)

            # grouped-weight combine: each gathered row is [v00, dx, dy, dxy]
            # blocks of 32ch. feats = (v0_0+v0_1+v0_2) + f0*(dx0+dx2)
            #   + f1*(dx1+dy0) + f2*(dy1+dy2) + f01*dxy0 + f12*dxy1 + f02*dxy2
            def blk(g, i):
                return g[:, :, i * 32:(i + 1) * 32]

            def wb(c):
                return wt3[:, :, c].unsqueeze(2).to_broadcast([128, KB, 32])

            tA = work.tile([128, KB, 32], bf16, tag="tA", name="tA")
            nc.vector.tensor_tensor(out=tA[:], in0=blk(gs[0], 1),
                                    in1=blk(gs[2], 1), op=add)
            tB = work.tile([128, KB, 32], bf16, tag="tB", name="tB")
            nc.vector.tensor_tensor(out=tB[:], in0=blk(gs[1], 1),
                                    in1=blk(gs[0], 2), op=add)
            tC = work.tile([128, KB, 32], bf16, tag="tC", name="tC")
            nc.vector.tensor_tensor(out=tC[:], in0=blk(gs[1], 2),
                                    in1=blk(gs[2], 2), op=add)
            acc = work.tile([128, KB, 32], bf16, tag="acc", name="acc")
            nc.vector.tensor_tensor(out=acc[:], in0=blk(gs[0], 0),
                                    in1=blk(gs[1], 0), op=add)
            nc.vector.tensor_tensor(out=acc[:], in0=acc[:],
                                    in1=blk(gs[2], 0), op=add)
            terms = ((tA[:], 0), (tB[:], 1), (tC[:], 2),
                     (blk(gs[0], 3), 3), (blk(gs[1], 3), 4),
                     (blk(gs[2], 3), 5))
            for src, c in terms:
                p = work.tile([128, KB, 32], bf16, tag="p", name="p", bufs=3)
                nc.vector.tensor_tensor(out=p[:], in0=src, in1=wb(c), op=mult)
                nc.vector.tensor_tensor(out=acc[:], in0=acc[:], in1=p[:],
                                        op=add)
            feats3 = acc[:]

            res = spool.tile([1, NB], bf16, tag="res", name="res", bufs=2)
            for hb in range(4):
                ftp = psum.tile([33, 1024], bf16, tag="ftp", name="ftp",
                                space="PSUM", bufs=2)
                for q in range(8):
                    k = hb * 8 + q
                    nc.tensor.transpose(
                        out=ftp[0:32, q * 128:(q + 1) * 128],
                        in_=feats3[:, k, :], identity=ident[:])
                fts = spool.tile([33, 1024], bf16, tag="fts", name="fts",
                                 bufs=2)
                nc.scalar.activation(fts[0:32, :], ftp[0:32, :], AF.Copy)

                for s in range(2):
                    fsl = fts[:, s * 512:(s + 1) * 512]
                    mm0 = psum.tile([HID, 512], f32, tag="mm", name="mm",
                                    space="PSUM", bufs=4)
                    nc.tensor.matmul(out=mm0[:], lhsT=w0s[:], rhs=fsl,
                                     start=True, stop=True)
                    h0 = work.tile([HID, 512], bf16, tag="h0", name="h0",
                                   bufs=3)
                    nc.scalar.activation(h0[:], mm0[:], AF.Relu)

                    mm1 = psum.tile([HID, 512], f32, tag="mm", name="mm",
                                    space="PSUM", bufs=4)
                    nc.tensor.matmul(out=mm1[:], lhsT=w1s[:], rhs=h0[:],
                                     start=True, stop=True)
                    h1 = work.tile([HID, 512], bf16, tag="h1", name="h1",
                                   bufs=3)
                    nc.scalar.activation(h1[:], mm1[:], AF.Relu,
                                         bias=b1s[:, 0:1])

                    mm2 = psum.tile([HID, 512], f32, tag="mm", name="mm",
                                    space="PSUM", bufs=4)
                    nc.tensor.matmul(out=mm2[:], lhsT=w2s[:], rhs=h1[:],
                                     start=True, stop=True)
                    h2w = work.tile([HID, 512], bf16, tag="h2w", name="h2w",
                                    bufs=3)
                    nc.vector.tensor_scalar(out=h2w[:], in0=mm2[:],
                                            scalar1=b2s[:, 0:1], scalar2=0.0,
                                            op0=add, op1=amax)

                    mm3 = psum.tile([1, 512], f32, tag="mm3", name="mm3",
                                    space="PSUM", bufs=2)
                    nc.tensor.matmul(out=mm3[:], lhsT=s3s[:], rhs=h2w[:],
                                     start=True, stop=True)
                    o0 = hb * 1024 + s * 512
                    if s == 0:
                        nc.vector.tensor_scalar_add(res[0:1, o0:o0 + 512],
                                                    mm3[:], b3s[0:1, 0:1])
                    else:
                        nc.scalar.activation(res[0:1, o0:o0 + 512], mm3[:],
                                             AF.Identity, bias=b3s[0:1, 0:1])

            nc.sync.dma_start(outv[:, b * NB:(b + 1) * NB], res[:])

    if do_finalize:
        nc.finalize()
    return nc


def _get_nc():
    if "nc" not in _BUILT:
        _BUILT["nc"] = _build_nc()
    return _BUILT["nc"]


def _build_patch_tables(planes: np.ndarray) -> np.ndarray:
    # planes [3, 32, 512, 512] -> finite-difference patch tables
    # PT[pl, cell, 4*32] bf16 with blocks [v00, dx, dy, dxy].
    import ml_dtypes

    p = planes.transpose(0, 2, 3, 1).astype(np.float32)  # [3, H, W, C]
    v01 = np.zeros_like(p)
    v01[:, :, :-1] = p[:, :, 1:]
    v10 = np.zeros_like(p)
    v10[:, :-1, :] = p[:, 1:, :]
    v11 = np.zeros_like(p)
    v11[:, :-1, :-1] = p[:, 1:, 1:]
    pt = np.empty((3, RES, RES, 4, EMB), dtype=np.float32)
    pt[:, :, :, 0] = p
    pt[:, :, :, 1] = v01 - p
    pt[:, :, :, 2] = v10 - p
    pt[:, :, :, 3] = v11 - v10 - v01 + p
    return np.ascontiguousarray(
        pt.reshape(3, CELLS, 4 * EMB)).astype(ml_dtypes.bfloat16)


def _prepare(inputs):
    import ml_dtypes

    bf = ml_dtypes.bfloat16
    coords = np.asarray(inputs["coordinates"], dtype=np.float32)
    n = coords.shape[0]

    pt = _build_patch_tables(np.asarray(inputs["planes"], np.float32))

    w0 = np.asarray(inputs["w0"], np.float32)
    w1 = np.asarray(inputs["w1"], np.float32)
    w2 = np.asarray(inputs["w2"], np.float32)
    w3 = np.asarray(inputs["w3"], np.float32).reshape(HID)
    b0 = np.asarray(inputs["b0"], np.float32)
    b1 = np.asarray(inputs["b1"], np.float32).reshape(HID, 1)
    b2 = np.asarray(inputs["b2"], np.float32).reshape(HID)
    b3 = np.asarray(inputs["b3"], np.float32).reshape(1, 1)

    w0c = np.ascontiguousarray(
        np.vstack([w0.T, b0.reshape(1, HID)])).astype(bf)   # [33, 128]
    w1c = np.ascontiguousarray(w1.T).astype(bf)
    w2p = w2 * np.abs(w3).reshape(HID, 1)                   # scale rows
    w2c = np.ascontiguousarray(w2p.T).astype(bf)
    s3c = np.sign(w3).reshape(HID, 1).astype(bf)
    b2c = (b2 * np.abs(w3)).reshape(HID, 1).astype(np.float32)

    # pixel coords, floors, fracs (mirrors reference: x = (c+1)*0.5*(R-1))
    pix = (coords + 1.0) * np.float32(0.5 * (RES - 1))
    p0 = np.floor(pix).astype(np.int64)
    fr = (pix - p0.astype(np.float32)).astype(np.float32)
    p0 = np.clip(p0, 0, RES - 1)

    gband = np.clip(p0[:, 1] >> 6, 0, NCORES - 1)
    jband = np.clip(p0[:, 2] >> 6, 0, NPB - 1)
    key = gband * NPB + jband
    order = np.argsort(key, kind="stable")
    counts = np.bincount(key, minlength=NCORES * NPB)
    assert counts.max() <= BCAP, f"bucket overflow: {counts.max()}"

    # per-point window-local indices and grouped weights, in original order
    idx0 = (p0[:, 1] - (gband << 6)) * RES + p0[:, 0]
    idx1 = (p0[:, 2] - (jband << 6)) * RES + p0[:, 1]
    idx2 = (p0[:, 2] - (jband << 6)) * RES + p0[:, 0]
    idx_all = np.stack([idx0, idx1, idx2], axis=1).astype(np.int16)
    f0, f1, f2 = fr[:, 0], fr[:, 1], fr[:, 2]
    w_all = np.stack([f0, f1, f2, f0 * f1, f1 * f2, f0 * f2],
                     axis=1).astype(bf)

    starts = np.concatenate(([0], np.cumsum(counts)))
    idx_s = np.zeros((NCORES * NPB, BCAP, 3), np.int16)
    w_s = np.zeros((NCORES * NPB, BCAP, 6), bf)
    ids = np.full((NCORES * NPB, BCAP), -1, np.int64)
    for kbkt in range(NCORES * NPB):
        sel = order[starts[kbkt]:starts[kbkt + 1]]
        idx_s[kbkt, :len(sel)] = idx_all[sel]
        w_s[kbkt, :len(sel)] = w_all[sel]
        ids[kbkt, :len(sel)] = sel

    in_maps = []
    for g in range(NCORES):
        ci = idx_s[g * NPB:(g + 1) * NPB].reshape(NP, 3)
        # [NBLK, 3, S, 16] -> wrap 16 partitions, tile x8 -> [128, NBLK*3*S]
        I = ci.reshape(NBLK, NB, 3).transpose(0, 2, 1)         # [NBLK, 3, NB]
        I = I.reshape(NBLK, 3, S, 16).transpose(0, 1, 3, 2)    # [NBLK,3,16,S]
        I = np.tile(I, (1, 1, 8, 1)).transpose(2, 0, 1, 3)     # [128,NBLK,3,S]
        I = np.ascontiguousarray(I).reshape(128, NBLK * 3 * S)

        cw = w_s[g * NPB:(g + 1) * NPB].reshape(NP, 6)
        W = cw.reshape(NBLK, KB, 128, 6).transpose(2, 0, 1, 3)
        W = np.ascontiguousarray(W).reshape(128, NBLK * KB * 6)

        in_maps.append({
            "pt0": np.ascontiguousarray(pt[0][g * WIN:(g + 1) * WIN]),
            "pt1": pt[1], "pt2": pt[2],
            "idx": I, "wgt": W,
            "w0c": w0c, "w1c": w1c, "w2c": w2c, "s3c": s3c,
            "b1c": b1, "b2c": b2c, "b3c": b3,
        })
    return in_maps, ids.reshape(NCORES, NP), n


def kernel(**inputs: np.ndarray) -> np.ndarray:
    global LAST_RESULTS
    from concourse.bass_utils import run_bass_kernel_spmd

    in_maps, flat_ids, n = _prepare(inputs)
    nc = _get_nc()
    LAST_RESULTS = run_bass_kernel_spmd(nc, in_maps, list(range(NCORES)))

    full = np.zeros(n, np.float32)
    for g in range(NCORES):
        o = np.asarray(LAST_RESULTS.results[g]["out"]).astype(np.float32)
        m = flat_ids[g] >= 0
        full[flat_ids[g][m]] = o[m]
    return full.reshape(1, n, 1).astype(np.float32)


# revision 20
# speedup vs baseline: 1.6148x; 1.0091x over previous
"""Triplane embedding-lookup + MLP kernel for Trainium2 (8 NeuronCores), v3.

Architecture (vs v2 baseline at ~1.37ms):
  - Host: bucket-sort points by (band(y0(c1)) -> core, band(y0(c2)) -> bucket),
    precompute window-local int16 cell indices (3 per point) and grouped
    bilinear weights (6 bf16 per point) on host.  Patch tables are stored in a
    finite-difference basis [v00, dx, dy, dxy] per cell, bf16 (256B rows).
  - Device per block (4096 pts): 6 SWDGE dma_gathers (2048 idxs, 256B elem),
    grouped-weight combine on DVE in bf16 (17 ops, exploits shared fractional
    parts across the 3 planes: feats = V0 + f0*A + f1*B + f2*C + f01*Dxy0 +
    f12*Dxy1 + f02*Dxy2), PE transpose per 128-pt group into PSUM, bias-folded
    K=33 L0 matmul, N=1024 bf16 matmul MLP with |w3| folded into W2's rows and
    sign(w3) as the M=1 output matmul, relu/evac split across ACT+DVE.
  - Host unsorts the output.
"""

import sys

sys.path.insert(0, "/opt/trn_rl_repo")

from contextlib import ExitStack

import numpy as np

RES = 512
CELLS = RES * RES
EMB = 32
HID = 128
N = 1_000_000
NCORES = 8

BROWS = 64           # plane rows per band
WIN = BROWS * RES    # 32768 rows per window (int16-addressable)
BCAP = 16384         # points per bucket (padded)
NPB = 8              # buckets per core
NP = NPB * BCAP      # 131072 points per core
NB = 4096            # points per block
KB = NB // 128       # 32 k-groups per block
NBLK = NP // NB      # 32 blocks per core
GQ = 1024            # idxs per dma_gather (65 descs/lane fits the SWDGE ring;
                     # 2048 -> 129/lane deadlocks the carveout ring on HW)
S = NB // 16         # 256 idx cols per plane per block

LAST_RESULTS = None
_BUILT = {}


def _build_nc(do_finalize: bool = True):
    from concourse import bacc, mybir
    import concourse.tile as tile
    from concourse.masks import make_identity

    dt = mybir.dt
    f32 = dt.float32
    i16 = dt.int16
    bf16 = dt.bfloat16
    add = mybir.AluOpType.add
    mult = mybir.AluOpType.mult
    amax = mybir.AluOpType.max
    AF = mybir.ActivationFunctionType

    nc = bacc.Bacc("TRN2", target_bir_lowering=False, num_swdge_queues=4,
                   dynamic_dma_scratch_size=65536)

    pt0d = nc.dram_tensor("pt0", [WIN, 128], bf16, kind="ExternalInput")
    pt1d = nc.dram_tensor("pt1", [NPB * WIN, 128], bf16, kind="ExternalInput")
    pt2d = nc.dram_tensor("pt2", [NPB * WIN, 128], bf16, kind="ExternalInput")
    idxd = nc.dram_tensor("idx", [128, NBLK * 3 * S], i16, kind="ExternalInput")
    wd = nc.dram_tensor("wgt", [128, NBLK * KB * 6], bf16, kind="ExternalInput")
    w0d = nc.dram_tensor("w0c", [33, HID], bf16, kind="ExternalInput")
    w1d = nc.dram_tensor("w1c", [HID, HID], bf16, kind="ExternalInput")
    w2d = nc.dram_tensor("w2c", [HID, HID], bf16, kind="ExternalInput")
    s3d = nc.dram_tensor("s3c", [HID, 1], bf16, kind="ExternalInput")
    b1d = nc.dram_tensor("b1c", [HID, 1], f32, kind="ExternalInput")
    b2d = nc.dram_tensor("b2c", [HID, 1], f32, kind="ExternalInput")
    b3d = nc.dram_tensor("b3c", [1, 1], f32, kind="ExternalInput")
    outd = nc.dram_tensor("out", [NP], bf16, kind="ExternalOutput")
    outv = outd[:].unsqueeze(0)

    with tile.TileContext(nc) as tc, ExitStack() as ctx:
        cpool = ctx.enter_context(tc.tile_pool(name="consts", bufs=1))

        def ct(shape, dtp, tag):
            return cpool.tile(shape, dtp, tag=tag, name=tag)

        io = ctx.enter_context(tc.tile_pool(name="io", bufs=3))
        gpool = ctx.enter_context(tc.tile_pool(name="gather", bufs=2))
        work = ctx.enter_context(tc.tile_pool(name="work", bufs=2))
        spool = ctx.enter_context(tc.tile_pool(name="stage", bufs=2))
        psum = ctx.enter_context(tc.tile_pool(name="psum", bufs=2, space="PSUM"))

        def load_streams(b):
            it = io.tile([128, 3 * S], i16, tag="idxt", name="idxt", bufs=3)
            nc.sync.dma_start(it[:], idxd[:, b * 3 * S:(b + 1) * 3 * S])
            w = io.tile([128, KB * 6], bf16, tag="wt", name="wt", bufs=3)
            nc.sync.dma_start(w[:], wd[:, b * KB * 6:(b + 1) * KB * 6])
            return it, w

        # prefetch the first blocks' streams so the gather pipeline starts
        # before the (longer) const-load tail drains from the HWDGE queue
        pre = {b: load_streams(b) for b in range(2)}

        w0s = ct([33, HID], bf16, "w0s")
        w1s = ct([HID, HID], bf16, "w1s")
        w2s = ct([HID, HID], bf16, "w2s")
        s3s = ct([HID, 1], bf16, "s3s")
        b1s = ct([HID, 1], f32, "b1s")
        b2s = ct([HID, 1], f32, "b2s")
        b3s = ct([1, 1], f32, "b3s")
        ident = ct([128, 128], bf16, "ident")
        for s, d in ((w0s, w0d), (w1s, w1d), (w2s, w2d), (s3s, s3d),
                     (b1s, b1d), (b2s, b2d), (b3s, b3d)):
            nc.sync.dma_start(s[:], d[:])
        make_identity(nc, ident[:])

        # fts buffers carry a constant ones-row (row 32) for the K=33
        # bias-folded L0; write it once per physical buffer before the loop.
        for _ in range(2):
            f = spool.tile([33, 1024], bf16, tag="fts", name="fts", bufs=2)
            nc.vector.memset(f[32:33, :], 1.0)

        for b in range(NBLK):
            j = b // (BCAP // NB)

            idxt, wt = pre.pop(b) if b in pre else load_streams(b)
            wt3 = wt[:].rearrange("p (k c) -> p k c", c=6)

            srcs = (pt0d[:],
                    pt1d[:][j * WIN:(j + 1) * WIN, :],
                    pt2d[:][j * WIN:(j + 1) * WIN, :])
            gs = []
            for pl in range(3):
                g = gpool.tile([128, KB, 128], bf16, tag=f"g{pl}",
                               name=f"g{pl}", bufs=2)
                for h in range(4):
                    nc.gpsimd.dma_gather(
                        out_ap=g[:, h * (KB // 4):(h + 1) * (KB // 4), :],
                        in_ap=srcs[pl],
                        idxs_ap=idxt[:, pl * S + h * (S // 4):
                                     pl * S + (h + 1) * (S // 4)],
                        num_idxs=GQ, num_idxs_reg=GQ, elem_size=128,
                        queue_num=(12 * b + 4 * pl + h) % 4)
                gs.append(g)

            # grouped-weight combine: each gathered row is [v00, dx, dy, dxy]
            # blocks of 32ch. feats = (v0_0+v0_1+v0_2) + f0*(dx0+dx2)
            #   + f1*(dx1+dy0) + f2*(dy1+dy2) + f01*dxy0 + f12*dxy1 + f02*dxy2
            def blk(g, i):
                return g[:, :, i * 32:(i + 1) * 32]

            def wb(c):
                return wt3[:, :, c].unsqueeze(2).to_broadcast([128, KB, 32])

            tA = work.tile([128, KB, 32], bf16, tag="tA", name="tA")
            nc.vector.tensor_tensor(out=tA[:], in0=blk(gs[0], 1),
                                    in1=blk(gs[2], 1), op=add)
            tB = work.tile([128, KB, 32], bf16, tag="tB", name="tB")
            nc.vector.tensor_tensor(out=tB[:], in0=blk(gs[1], 1),
                                    in1=blk(gs[0], 2), op=add)
            tC = work.tile([128, KB, 32], bf16, tag="tC", name="tC")
            nc.vector.tensor_tensor(out=tC[:], in0=blk(gs[1], 2),
                                    in1=blk(gs[2], 2), op=add)
            acc = work.tile([128, KB, 32], bf16, tag="acc", name="acc")
            nc.vector.tensor_tensor(out=acc[:], in0=blk(gs[0], 0),
                                    in1=blk(gs[1], 0), op=add)
            nc.vector.tensor_tensor(out=acc[:], in0=acc[:],
                                    in1=blk(gs[2], 0), op=add)
            terms = ((tA[:], 0), (tB[:], 1), (tC[:], 2),
                     (blk(gs[0], 3), 3), (blk(gs[1], 3), 4),
                     (blk(gs[2], 3), 5))
            for src, c in terms:
                p = work.tile([128, KB, 32], bf16, tag="p", name="p", bufs=3)
                nc.vector.tensor_tensor(out=p[:], in0=src, in1=wb(c), op=mult)
                nc.vector.tensor_tensor(out=acc[:], in0=acc[:], in1=p[:],
                                        op=add)
            feats3 = acc[:]

            res = spool.tile([1, NB], bf16, tag="res", name="res", bufs=2)
            for hb in range(4):
                ftp = psum.tile([33, 1024], bf16, tag="ftp", name="ftp",
                                space="PSUM", bufs=2)
                for q in range(8):
                    k = hb * 8 + q
                    nc.tensor.transpose(
                        out=ftp[0:32, q * 128:(q + 1) * 128],
                        in_=feats3[:, k, :], identity=ident[:])
                fts = spool.tile([33, 1024], bf16, tag="fts", name="fts",
                                 bufs=2)
                nc.scalar.activation(fts[0:32, :], ftp[0:32, :], AF.Copy)

                for s in range(2):
                    fsl = fts[:, s * 512:(s + 1) * 512]
                    mm0 = psum.tile([HID, 512], f32, tag="mm", name="mm",
                                    space="PSUM", bufs=4)
                    nc.tensor.matmul(out=mm0[:], lhsT=w0s[:], rhs=fsl,
                                     start=True, stop=True)
                    h0 = work.tile([HID, 512], bf16, tag="h0", name="h0",
                                   bufs=3)
                    nc.scalar.activation(h0[:], mm0[:], AF.Relu)

                    mm1 = psum.tile([HID, 512], f32, tag="mm", name="mm",
                                    space="PSUM", bufs=4)
                    nc.tensor.matmul(out=mm1[:], lhsT=w1s[:], rhs=h0[:],
                                     start=True, stop=True)
                    h1 = work.tile([HID, 512], bf16, tag="h1", name="h1",
                                   bufs=3)
                    nc.scalar.activation(h1[:], mm1[:], AF.Relu,
                                         bias=b1s[:, 0:1])

                    mm2 = psum.tile([HID, 512], f32, tag="mm", name="mm",
                                    space="PSUM", bufs=4)
                    nc.tensor.matmul(out=mm2[:], lhsT=w2s[:], rhs=h1[:],
                                     start=True, stop=True)
                    h2w = work.tile([HID, 512], bf16, tag="h2w", name="h2w",
                                    bufs=3)
                    nc.vector.tensor_scalar(out=h2w[:], in0=mm2[:],
                                            scalar1=b2s[:, 0:1], scalar2=0.0,
                                            op0=add, op1=amax)

                    mm3 = psum.tile([1, 512], f32, tag="mm3", name="mm3",
                                    space="PSUM", bufs=2)
                    nc.tensor.matmul(out=mm3[:], lhsT=s3s[:], rhs=h2w[:],
                                     start=True, stop=True)
                    o0 = hb * 1024 + s * 512
                    if s == 0:
                        nc.vector.tensor_scalar_add(res[0:1, o0:o0 + 512],
                                                    mm3[:], b3s[0:1, 0:1])
                    else:
                        nc.scalar.activation(res[0:1, o0:o0 + 512], mm3[:],
                                             AF.Identity, bias=b3s[0:1, 0:1])

            nc.sync.dma_start(outv[:, b * NB:(b + 1) * NB], res[:])

    if do_finalize:
        nc.finalize()
    return nc


def _get_nc():
    if "nc" not in _BUILT:
        _BUILT["nc"] = _build_nc()
    return _BUILT["nc"]


def _build_patch_tables(planes: np.ndarray) -> np.ndarray:
    # planes [3, 32, 512, 512] -> finite-difference patch tables
    # PT[pl, cell, 4*32] bf16 with blocks [v00, dx, dy, dxy].
    import ml_dtypes

    p = planes.transpose(0, 2, 3, 1).astype(np.float32)  # [3, H, W, C]
    v01 = np.zeros_like(p)
    v01[:, :, :-1] = p[:, :, 1:]
    v10 = np.zeros_like(p)
    v10[:, :-1, :] = p[:, 1:, :]
    v11 = np.zeros_like(p)
    v11[:, :-1, :-1] = p[:, 1:, 1:]
    pt = np.empty((3, RES, RES, 4, EMB), dtype=np.float32)
    pt[:, :, :, 0] = p
    pt[:, :, :, 1] = v01 - p
    pt[:, :, :, 2] = v10 - p
    pt[:, :, :, 3] = v11 - v10 - v01 + p
    return np.ascontiguousarray(
        pt.reshape(3, CELLS, 4 * EMB)).astype(ml_dtypes.bfloat16)


def _prepare(inputs):
    import ml_dtypes

    bf = ml_dtypes.bfloat16
    coords = np.asarray(inputs["coordinates"], dtype=np.float32)
    n = coords.shape[0]

    pt = _build_patch_tables(np.asarray(inputs["planes"], np.float32))

    w0 = np.asarray(inputs["w0"], np.float32)
    w1 = np.asarray(inputs["w1"], np.float32)
    w2 = np.asarray(inputs["w2"], np.float32)
    w3 = np.asarray(inputs["w3"], np.float32).reshape(HID)
    b0 = np.asarray(inputs["b0"], np.float32)
    b1 = np.asarray(inputs["b1"], np.float32).reshape(HID, 1)
    b2 = np.asarray(inputs["b2"], np.float32).reshape(HID)
    b3 = np.asarray(inputs["b3"], np.float32).reshape(1, 1)

    w0c = np.ascontiguousarray(
        np.vstack([w0.T, b0.reshape(1, HID)])).astype(bf)   # [33, 128]
    w1c = np.ascontiguousarray(w1.T).astype(bf)
    w2p = w2 * np.abs(w3).reshape(HID, 1)                   # scale rows
    w2c = np.ascontiguousarray(w2p.T).astype(bf)
    s3c = np.sign(w3).reshape(HID, 1).astype(bf)
    b2c = (b2 * np.abs(w3)).reshape(HID, 1).astype(np.float32)

    # pixel coords, floors, fracs (mirrors reference: x = (c+1)*0.5*(R-1))
    pix = (coords + 1.0) * np.float32(0.5 * (RES - 1))
    p0 = np.floor(pix).astype(np.int64)
    fr = (pix - p0.astype(np.float32)).astype(np.float32)
    p0 = np.clip(p0, 0, RES - 1)

    gband = np.clip(p0[:, 1] >> 6, 0, NCORES - 1)
    jband = np.clip(p0[:, 2] >> 6, 0, NPB - 1)
    key = gband * NPB + jband
    order = np.argsort(key, kind="stable")
    counts = np.bincount(key, minlength=NCORES * NPB)
    assert counts.max() <= BCAP, f"bucket overflow: {counts.max()}"

    # per-point window-local indices and grouped weights, in original order
    idx0 = (p0[:, 1] - (gband << 6)) * RES + p0[:, 0]
    idx1 = (p0[:, 2] - (jband << 6)) * RES + p0[:, 1]
    idx2 = (p0[:, 2] - (jband << 6)) * RES + p0[:, 0]
    idx_all = np.stack([idx0, idx1, idx2], axis=1).astype(np.int16)
    f0, f1, f2 = fr[:, 0], fr[:, 1], fr[:, 2]
    w_all = np.stack([f0, f1, f2, f0 * f1, f1 * f2, f0 * f2],
                     axis=1).astype(bf)

    starts = np.concatenate(([0], np.cumsum(counts)))
    idx_s = np.zeros((NCORES * NPB, BCAP, 3), np.int16)
    w_s = np.zeros((NCORES * NPB, BCAP, 6), bf)
    ids = np.full((NCORES * NPB, BCAP), -1, np.int64)
    for kbkt in range(NCORES * NPB):
        sel = order[starts[kbkt]:starts[kbkt + 1]]
        idx_s[kbkt, :len(sel)] = idx_all[sel]
        w_s[kbkt, :len(sel)] = w_all[sel]
        ids[kbkt, :len(sel)] = sel

    in_maps = []
    for g in range(NCORES):
        ci = idx_s[g * NPB:(g + 1) * NPB].reshape(NP, 3)
        # [NBLK, 3, S, 16] -> wrap 16 partitions, tile x8 -> [128, NBLK*3*S]
        I = ci.reshape(NBLK, NB, 3).transpose(0, 2, 1)         # [NBLK, 3, NB]
        I = I.reshape(NBLK, 3, S, 16).transpose(0, 1, 3, 2)    # [NBLK,3,16,S]
        I = np.tile(I, (1, 1, 8, 1)).transpose(2, 0, 1, 3)     # [128,NBLK,3,S]
        I = np.ascontiguousarray(I).reshape(128, NBLK * 3 * S)

        cw = w_s[g * NPB:(g + 1) * NPB].reshape(NP, 6)
        W = cw.reshape(NBLK, KB, 128, 6).transpose(2, 0, 1, 3)
        W = np.ascontiguousarray(W).reshape(128, NBLK * KB * 6)

        in_maps.append({
            "pt0": np.ascontiguousarray(pt[0][g * WIN:(g + 1) * WIN]),
            "pt1": pt[1], "pt2": pt[2],
            "idx": I, "wgt": W,
            "w0c": w0c, "w1c": w1c, "w2c": w2c, "s3c": s3c,
            "b1c": b1, "b2c": b2c, "b3c": b3,
        })
    return in_maps, ids.reshape(NCORES, NP), n


def kernel(**inputs: np.ndarray) -> np.ndarray:
    global LAST_RESULTS
    from concourse.bass_utils import run_bass_kernel_spmd

    in_maps, flat_ids, n = _prepare(inputs)
    nc = _get_nc()
    LAST_RESULTS = run_bass_kernel_spmd(nc, in_maps, list(range(NCORES)))

    full = np.zeros(n, np.float32)
    for g in range(NCORES):
        o = np.asarray(LAST_RESULTS.results[g]["out"]).astype(np.float32)
        m = flat_ids[g] >= 0
        full[flat_ids[g][m]] = o[m]
    return full.reshape(1, n, 1).astype(np.float32)


# revision 22
# speedup vs baseline: 1.6817x; 1.0414x over previous
"""Triplane embedding-lookup + MLP kernel for Trainium2 (8 NeuronCores), v3.

Architecture (vs v2 baseline at ~1.37ms):
  - Host: bucket-sort points by (band(y0(c1)) -> core, band(y0(c2)) -> bucket),
    precompute window-local int16 cell indices (3 per point) and grouped
    bilinear weights (6 bf16 per point) on host.  Patch tables are stored in a
    finite-difference basis [v00, dx, dy, dxy] per cell, bf16 (256B rows).
  - Device per block (4096 pts): 6 SWDGE dma_gathers (2048 idxs, 256B elem),
    grouped-weight combine on DVE in bf16 (17 ops, exploits shared fractional
    parts across the 3 planes: feats = V0 + f0*A + f1*B + f2*C + f01*Dxy0 +
    f12*Dxy1 + f02*Dxy2), PE transpose per 128-pt group into PSUM, bias-folded
    K=33 L0 matmul, N=1024 bf16 matmul MLP with |w3| folded into W2's rows and
    sign(w3) as the M=1 output matmul, relu/evac split across ACT+DVE.
  - Host unsorts the output.
"""

import sys

sys.path.insert(0, "/opt/trn_rl_repo")

from contextlib import ExitStack

import numpy as np

RES = 512
CELLS = RES * RES
EMB = 32
HID = 128
N = 1_000_000
NCORES = 8

BROWS = 64           # plane rows per band
WIN = BROWS * RES    # 32768 rows per window (int16-addressable)
BCAP = 16384         # points per bucket (padded)
NPB = 8              # buckets per core
NP = NPB * BCAP      # 131072 points per core
NB = 4096            # points per block
KB = NB // 128       # 32 k-groups per block
NBLK = NP // NB      # 32 blocks per core
GQ = 1024            # idxs per dma_gather (65 descs/lane fits the SWDGE ring;
                     # 2048 -> 129/lane deadlocks the carveout ring on HW)
S = NB // 16         # 256 idx cols per plane per block

LAST_RESULTS = None
_BUILT = {}


def _build_nc(do_finalize: bool = True):
    from concourse import bacc, mybir
    import concourse.tile as tile
    from concourse.masks import make_identity

    dt = mybir.dt
    f32 = dt.float32
    i16 = dt.int16
    bf16 = dt.bfloat16
    add = mybir.AluOpType.add
    mult = mybir.AluOpType.mult
    amax = mybir.AluOpType.max
    AF = mybir.ActivationFunctionType

    nc = bacc.Bacc("TRN2", target_bir_lowering=False, num_swdge_queues=4,
                   dynamic_dma_scratch_size=65536)

    pt0d = nc.dram_tensor("pt0", [WIN, 128], bf16, kind="ExternalInput")
    pt1d = nc.dram_tensor("pt1", [NPB * WIN, 128], bf16, kind="ExternalInput")
    pt2d = nc.dram_tensor("pt2", [NPB * WIN, 128], bf16, kind="ExternalInput")
    idxd = nc.dram_tensor("idx", [128, NBLK * 3 * S], i16, kind="ExternalInput")
    wd = nc.dram_tensor("wgt", [128, NBLK * KB * 6], bf16, kind="ExternalInput")
    w0d = nc.dram_tensor("w0c", [33, HID], bf16, kind="ExternalInput")
    w1d = nc.dram_tensor("w1c", [HID, HID], bf16, kind="ExternalInput")
    w2d = nc.dram_tensor("w2c", [HID, HID], bf16, kind="ExternalInput")
    s3d = nc.dram_tensor("s3c", [HID, 1], bf16, kind="ExternalInput")
    b1d = nc.dram_tensor("b1c", [HID, 1], f32, kind="ExternalInput")
    b2d = nc.dram_tensor("b2c", [HID, 1], f32, kind="ExternalInput")
    b3d = nc.dram_tensor("b3c", [1, 1], f32, kind="ExternalInput")
    outd = nc.dram_tensor("out", [NP], bf16, kind="ExternalOutput")
    outv = outd[:].unsqueeze(0)

    with tile.TileContext(nc) as tc, ExitStack() as ctx:
        cpool = ctx.enter_context(tc.tile_pool(name="consts", bufs=1))

        def ct(shape, dtp, tag):
            return cpool.tile(shape, dtp, tag=tag, name=tag)

        io = ctx.enter_context(tc.tile_pool(name="io", bufs=3))
        gpool = ctx.enter_context(tc.tile_pool(name="gather", bufs=2))
        work = ctx.enter_context(tc.tile_pool(name="work", bufs=2))
        spool = ctx.enter_context(tc.tile_pool(name="stage", bufs=2))
        psum = ctx.enter_context(tc.tile_pool(name="psum", bufs=2, space="PSUM"))

        def load_streams(b):
            it = io.tile([128, 3 * S], i16, tag="idxt", name="idxt", bufs=3)
            nc.sync.dma_start(it[:], idxd[:, b * 3 * S:(b + 1) * 3 * S])
            w = io.tile([128, KB * 6], bf16, tag="wt", name="wt", bufs=3)
            nc.sync.dma_start(w[:], wd[:, b * KB * 6:(b + 1) * KB * 6])
            return it, w

        # prefetch the first blocks' streams so the gather pipeline starts
        # before the (longer) const-load tail drains from the HWDGE queue
        pre = {b: load_streams(b) for b in range(2)}

        w0s = ct([33, HID], bf16, "w0s")
        w1s = ct([HID, HID], bf16, "w1s")
        w2s = ct([HID, HID], bf16, "w2s")
        s3s = ct([HID, 1], bf16, "s3s")
        b1s = ct([HID, 1], f32, "b1s")
        b2s = ct([HID, 1], f32, "b2s")
        b3s = ct([1, 1], f32, "b3s")
        ident = ct([128, 128], bf16, "ident")
        for s, d in ((w0s, w0d), (w1s, w1d), (w2s, w2d), (s3s, s3d),
                     (b1s, b1d), (b2s, b2d), (b3s, b3d)):
            nc.sync.dma_start(s[:], d[:])
        make_identity(nc, ident[:])

        # fts buffers carry a constant ones-row (row 32) for the K=33
        # bias-folded L0; write it once per physical buffer before the loop.
        for _ in range(2):
            f = spool.tile([33, 1024], bf16, tag="fts", name="fts", bufs=2)
            nc.vector.memset(f[32:33, :], 1.0)

        for b in range(NBLK):
            j = b // (BCAP // NB)

            idxt, wt = pre.pop(b) if b in pre else load_streams(b)
            wt3 = wt[:].rearrange("p (k c) -> p k c", c=6)

            srcs = (pt0d[:],
                    pt1d[:][j * WIN:(j + 1) * WIN, :],
                    pt2d[:][j * WIN:(j + 1) * WIN, :])
            gs = []
            for pl in range(3):
                g = gpool.tile([128, KB, 128], bf16, tag=f"g{pl}",
                               name=f"g{pl}", bufs=2)
                gs.append(g)
            # h-major emission: the first half-block's chunks (h=0,1) of all
            # 3 planes land first, so its combine/MLP starts after 6 of 12
            # gathers -- halves the post-gather pipeline drain
            for h in range(4):
                for pl in range(3):
                    nc.gpsimd.dma_gather(
                        out_ap=gs[pl][:, h * (KB // 4):(h + 1) * (KB // 4), :],
                        in_ap=srcs[pl],
                        idxs_ap=idxt[:, pl * S + h * (S // 4):
                                     pl * S + (h + 1) * (S // 4)],
                        num_idxs=GQ, num_idxs_reg=GQ, elem_size=128,
                        queue_num=(12 * b + 3 * h + pl) % 4)

            res = spool.tile([1, NB], bf16, tag="res", name="res", bufs=2)
            HK = KB // 2
            for half in range(2):
                k0 = half * HK

                # grouped-weight combine (per half-block): each gathered row
                # is [v00, dx, dy, dxy] blocks of 32ch.
                # feats = (v0_0+v0_1+v0_2) + f0*(dx0+dx2) + f1*(dx1+dy0)
                #         + f2*(dy1+dy2) + f01*dxy0 + f12*dxy1 + f02*dxy2
                def blk(g, i):
                    return g[:, k0:k0 + HK, i * 32:(i + 1) * 32]

                def wb(c):
                    return wt3[:, k0:k0 + HK, c].unsqueeze(2).to_broadcast(
                        [128, HK, 32])

                tA = work.tile([128, HK, 32], bf16, tag="tA", name="tA",
                               bufs=4)
                nc.vector.tensor_tensor(out=tA[:], in0=blk(gs[0], 1),
                                        in1=blk(gs[2], 1), op=add)
                tB = work.tile([128, HK, 32], bf16, tag="tB", name="tB",
                               bufs=4)
                nc.vector.tensor_tensor(out=tB[:], in0=blk(gs[1], 1),
                                        in1=blk(gs[0], 2), op=add)
                tC = work.tile([128, HK, 32], bf16, tag="tC", name="tC",
                               bufs=4)
                nc.vector.tensor_tensor(out=tC[:], in0=blk(gs[1], 2),
                                        in1=blk(gs[2], 2), op=add)
                acc = work.tile([128, HK, 32], bf16, tag="acc", name="acc",
                                bufs=4)
                nc.vector.tensor_tensor(out=acc[:], in0=blk(gs[0], 0),
                                        in1=blk(gs[1], 0), op=add)
                nc.vector.tensor_tensor(out=acc[:], in0=acc[:],
                                        in1=blk(gs[2], 0), op=add)
                terms = ((tA[:], 0), (tB[:], 1), (tC[:], 2),
                         (blk(gs[0], 3), 3), (blk(gs[1], 3), 4),
                         (blk(gs[2], 3), 5))
                for src, c in terms:
                    p = work.tile([128, HK, 32], bf16, tag="p", name="p",
                                  bufs=4)
                    nc.vector.tensor_tensor(out=p[:], in0=src, in1=wb(c),
                                            op=mult)
                    nc.vector.tensor_tensor(out=acc[:], in0=acc[:],
                                            in1=p[:], op=add)
                feats3 = acc[:]

                for hb in (2 * half, 2 * half + 1):
                    ftp = psum.tile([33, 1024], bf16, tag="ftp", name="ftp",
                                    space="PSUM", bufs=2)
                    for q in range(8):
                        k = hb * 8 + q
                        nc.tensor.transpose(
                            out=ftp[0:32, q * 128:(q + 1) * 128],
                            in_=feats3[:, k - k0, :], identity=ident[:])
                    fts = spool.tile([33, 1024], bf16, tag="fts", name="fts",
                                     bufs=2)
                    nc.scalar.activation(fts[0:32, :], ftp[0:32, :], AF.Copy)

                    for s in range(2):
                        fsl = fts[:, s * 512:(s + 1) * 512]
                        mm0 = psum.tile([HID, 512], f32, tag="mm", name="mm",
                                        space="PSUM", bufs=4)
                        nc.tensor.matmul(out=mm0[:], lhsT=w0s[:], rhs=fsl,
                                         start=True, stop=True)
                        h0 = work.tile([HID, 512], bf16, tag="h0", name="h0",
                                       bufs=3)
                        nc.scalar.activation(h0[:], mm0[:], AF.Relu)

                        mm1 = psum.tile([HID, 512], f32, tag="mm", name="mm",
                                        space="PSUM", bufs=4)
                        nc.tensor.matmul(out=mm1[:], lhsT=w1s[:], rhs=h0[:],
                                         start=True, stop=True)
                        h1 = work.tile([HID, 512], bf16, tag="h1", name="h1",
                                       bufs=3)
                        nc.scalar.activation(h1[:], mm1[:], AF.Relu,
                                             bias=b1s[:, 0:1])

                        mm2 = psum.tile([HID, 512], f32, tag="mm", name="mm",
                                        space="PSUM", bufs=4)
                        nc.tensor.matmul(out=mm2[:], lhsT=w2s[:], rhs=h1[:],
                                         start=True, stop=True)
                        h2w = work.tile([HID, 512], bf16, tag="h2w",
                                        name="h2w", bufs=3)
                        nc.vector.tensor_scalar(out=h2w[:], in0=mm2[:],
                                                scalar1=b2s[:, 0:1],
                                                scalar2=0.0,
                                                op0=add, op1=amax)

                        mm3 = psum.tile([1, 512], f32, tag="mm3", name="mm3",
                                        space="PSUM", bufs=2)
                        nc.tensor.matmul(out=mm3[:], lhsT=s3s[:], rhs=h2w[:],
                                         start=True, stop=True)
                        o0 = hb * 1024 + s * 512
                        if s == 0:
                            nc.vector.tensor_scalar_add(
                                res[0:1, o0:o0 + 512], mm3[:], b3s[0:1, 0:1])
                        else:
                            nc.scalar.activation(
                                res[0:1, o0:o0 + 512], mm3[:],
                                AF.Identity, bias=b3s[0:1, 0:1])

            nc.sync.dma_start(outv[:, b * NB:(b + 1) * NB], res[:])

    if do_finalize:
        nc.finalize()
    return nc


def _get_nc():
    if "nc" not in _BUILT:
        _BUILT["nc"] = _build_nc()
    return _BUILT["nc"]


def _build_patch_tables(planes: np.ndarray) -> np.ndarray:
    # planes [3, 32, 512, 512] -> finite-difference patch tables
    # PT[pl, cell, 4*32] bf16 with blocks [v00, dx, dy, dxy].
    import ml_dtypes

    p = planes.transpose(0, 2, 3, 1).astype(np.float32)  # [3, H, W, C]
    v01 = np.zeros_like(p)
    v01[:, :, :-1] = p[:, :, 1:]
    v10 = np.zeros_like(p)
    v10[:, :-1, :] = p[:, 1:, :]
    v11 = np.zeros_like(p)
    v11[:, :-1, :-1] = p[:, 1:, 1:]
    pt = np.empty((3, RES, RES, 4, EMB), dtype=np.float32)
    pt[:, :, :, 0] = p
    pt[:, :, :, 1] = v01 - p
    pt[:, :, :, 2] = v10 - p
    pt[:, :, :, 3] = v11 - v10 - v01 + p
    return np.ascontiguousarray(
        pt.reshape(3, CELLS, 4 * EMB)).astype(ml_dtypes.bfloat16)


def _prepare(inputs):
    import ml_dtypes

    bf = ml_dtypes.bfloat16
    coords = np.asarray(inputs["coordinates"], dtype=np.float32)
    n = coords.shape[0]

    pt = _build_patch_tables(np.asarray(inputs["planes"], np.float32))

    w0 = np.asarray(inputs["w0"], np.float32)
    w1 = np.asarray(inputs["w1"], np.float32)
    w2 = np.asarray(inputs["w2"], np.float32)
    w3 = np.asarray(inputs["w3"], np.float32).reshape(HID)
    b0 = np.asarray(inputs["b0"], np.float32)
    b1 = np.asarray(inputs["b1"], np.float32).reshape(HID, 1)
    b2 = np.asarray(inputs["b2"], np.float32).reshape(HID)
    b3 = np.asarray(inputs["b3"], np.float32).reshape(1, 1)

    w0c = np.ascontiguousarray(
        np.vstack([w0.T, b0.reshape(1, HID)])).astype(bf)   # [33, 128]
    w1c = np.ascontiguousarray(w1.T).astype(bf)
    w2p = w2 * np.abs(w3).reshape(HID, 1)                   # scale rows
    w2c = np.ascontiguousarray(w2p.T).astype(bf)
    s3c = np.sign(w3).reshape(HID, 1).astype(bf)
    b2c = (b2 * np.abs(w3)).reshape(HID, 1).astype(np.float32)

    # pixel coords, floors, fracs (mirrors reference: x = (c+1)*0.5*(R-1))
    pix = (coords + 1.0) * np.float32(0.5 * (RES - 1))
    p0 = np.floor(pix).astype(np.int64)
    fr = (pix - p0.astype(np.float32)).astype(np.float32)
    p0 = np.clip(p0, 0, RES - 1)

    gband = np.clip(p0[:, 1] >> 6, 0, NCORES - 1)
    jband = np.clip(p0[:, 2] >> 6, 0, NPB - 1)
    key = gband * NPB + jband
    order = np.argsort(key, kind="stable")
    counts = np.bincount(key, minlength=NCORES * NPB)
    assert counts.max() <= BCAP, f"bucket overflow: {counts.max()}"

    # per-point window-local indices and grouped weights, in original order
    idx0 = (p0[:, 1] - (gband << 6)) * RES + p0[:, 0]
    idx1 = (p0[:, 2] - (jband << 6)) * RES + p0[:, 1]
    idx2 = (p0[:, 2] - (jband << 6)) * RES + p0[:, 0]
    idx_all = np.stack([idx0, idx1, idx2], axis=1).astype(np.int16)
    f0, f1, f2 = fr[:, 0], fr[:, 1], fr[:, 2]
    w_all = np.stack([f0, f1, f2, f0 * f1, f1 * f2, f0 * f2],
                     axis=1).astype(bf)

    starts = np.concatenate(([0], np.cumsum(counts)))
    idx_s = np.zeros((NCORES * NPB, BCAP, 3), np.int16)
    w_s = np.zeros((NCORES * NPB, BCAP, 6), bf)
    ids = np.full((NCORES * NPB, BCAP), -1, np.int64)
    for kbkt in range(NCORES * NPB):
        sel = order[starts[kbkt]:starts[kbkt + 1]]
        idx_s[kbkt, :len(sel)] = idx_all[sel]
        w_s[kbkt, :len(sel)] = w_all[sel]
        ids[kbkt, :len(sel)] = sel

    in_maps = []
    for g in range(NCORES):
        ci = idx_s[g * NPB:(g + 1) * NPB].reshape(NP, 3)
        # [NBLK, 3, S, 16] -> wrap 16 partitions, tile x8 -> [128, NBLK*3*S]
        I = ci.reshape(NBLK, NB, 3).transpose(0, 2, 1)         # [NBLK, 3, NB]
        I = I.reshape(NBLK, 3, S, 16).transpose(0, 1, 3, 2)    # [NBLK,3,16,S]
        I = np.tile(I, (1, 1, 8, 1)).transpose(2, 0, 1, 3)     # [128,NBLK,3,S]
        I = np.ascontiguousarray(I).reshape(128, NBLK * 3 * S)

        cw = w_s[g * NPB:(g + 1) * NPB].reshape(NP, 6)
        W = cw.reshape(NBLK, KB, 128, 6).transpose(2, 0, 1, 3)
        W = np.ascontiguousarray(W).reshape(128, NBLK * KB * 6)

        in_maps.append({
            "pt0": np.ascontiguousarray(pt[0][g * WIN:(g + 1) * WIN]),
            "pt1": pt[1], "pt2": pt[2],
            "idx": I, "wgt": W,
            "w0c": w0c, "w1c": w1c, "w2c": w2c, "s3c": s3c,
            "b1c": b1, "b2c": b2c, "b3c": b3,
        })
    return in_maps, ids.reshape(NCORES, NP), n


def kernel(**inputs: np.ndarray) -> np.ndarray:
    global LAST_RESULTS
    from concourse.bass_utils import run_bass_kernel_spmd

    in_maps, flat_ids, n = _prepare(inputs)
    nc = _get_nc()
    LAST_RESULTS = run_bass_kernel_spmd(nc, in_maps, list(range(NCORES)))

    full = np.zeros(n, np.float32)
    for g in range(NCORES):
        o = np.asarray(LAST_RESULTS.results[g]["out"]).astype(np.float32)
        m = flat_ids[g] >= 0
        full[flat_ids[g][m]] = o[m]
    return full.reshape(1, n, 1).astype(np.float32)
